# revision 13
# baseline (speedup 1.0000x reference)
"""Trainium2 Bass kernel for nn_CannyDetector (B=8, Cin=3, C=8, H=W=512).

Strategy (pure data parallel, one image per NeuronCore, 8 cores):

Pipeline per core (see reference):
    h  = Wexp @ x; g = Pg @ gauss3x3(h); gx = Px @ sobelx(g); gy = Py @ sobely(g)
    mag = sqrt(gx^2+gy^2+eps); s = sum_c mag; dirs_k = s - shift_k(s)
    idx = round(atan2(gy,gx)/45deg); nms = mag * (dirs[idx]>0 & dirs[idx+4]>0)
    thr = step functions of mag; m = Wmerge @ thr; out = hysteresis(m)

All depthwise stencils and 1x1 channel mixes are linear, regrouped as:
3-tap horizontal FIRs on the 3 input channels (VectorE, all ops in
aligned bf16 2x/4x perf modes; the shared gauss-H row is computed once
and duplicated across partition slots by a small SBUF->SBUF DMA), then a
TensorE pass applying 5-tap vertical FIRs + 3->8 channel mixes +
a +22.5deg rotation, emitting p1,p2,q1=(p1+p2)/sqrt2 per channel.
Squares (ScalarE) give mag2 = p1^2+p2^2 and NMS sector bits as sign
tests beta=[p1^2>=p2^2], alpha=[2*q1^2>=mag2] (bit-exact vs
round(atan2)%4). The double threshold reduces to step functions of mag2.
A second TensorE pass contracts channels with merge weights into 4
axis-plane C_k and s.  The 1-channel tail runs on packed [128, H/128, W]
planes: row+-1 shifted planes come from TensorE shift-matrix matmuls
(not bulk DMA), column shifts from free-dim offsets of padded tiles,
so the tail's critical path is ~20 cheap bf16 VectorE ops.
"""

import math
import sys

import numpy as np

if "/opt/trn_rl_repo" not in sys.path:
    sys.path.insert(0, "/opt/trn_rl_repo")

import ml_dtypes

import concourse.bass as bass
import concourse.bacc as bacc
import concourse.tile as tile
from concourse import mybir
from concourse.bass_utils import run_bass_kernel_spmd

F32 = mybir.dt.float32
BF16 = mybir.dt.bfloat16
OP = mybir.AluOpType
AF = mybir.ActivationFunctionType

EPS = 1e-10
NEG_BIG = -1.0e30

R = 16          # chunk rows (PE pass granularity)
KROWS = R + 4   # moving-operand rows per chunk (5-tap vertical halo)
SCH = 4         # chunks per stripe
SROWS = R * SCH  # rows per stripe
SLOT_BASE = [0, 20, 40, 64, 84, 104]   # partition base per (qx0..2, qy0..2)
NPART = 124     # moving-operand partitions (rows 60..63 are zero padding)


def make_weights(w_expand, w_perm_gauss, w_perm_sx, w_perm_sy, w_merge,
                 low_t, high_t, H):
    """Host-side prep of stationary matrices and per-partition vectors."""
    e = math.exp(-1.0 / 18.0)
    norm = 1.0 / (2.0 * e + 1.0)
    # DVE computes xb = center + e*(left+right) = (2e+1)*gaussH; fold norm here
    phi_h = norm

    Ax = (w_perm_sx @ w_perm_gauss @ w_expand).astype(np.float64)   # (8,3)
    Ay = (w_perm_sy @ w_perm_gauss @ w_expand).astype(np.float64)

    gv = np.array([e, 1.0, e]) * norm            # vertical gauss (offsets -1..1)
    svx = np.array([0.5, 1.0, 0.5])              # sobel-x vertical
    svy = np.array([-1.0, 0.0, 1.0])             # sobel-y vertical
    VX = np.convolve(svx, gv) * phi_h            # offsets -2..2
    VY = np.convolve(svy, gv) * phi_h * 0.5      # qy carries 2x, fold 0.5

    cth, sth = math.cos(math.pi / 8), math.sin(math.pi / 8)
    rt = math.sqrt(0.5)
    comps = [
        (cth, -sth),                  # p1
        (sth, cth),                   # p2
        ((cth + sth) * rt, (cth - sth) * rt),  # q1 = (p1+p2)/sqrt2
    ]

    CH = H // R
    # stationaries [9, NPART, 128]: comp-major, pos in {top(c=0), mid, bot}
    wst = np.zeros((9, NPART, 128), np.float64)
    pos_chunk = {0: 0, 1: min(1, CH - 1), 2: CH - 1}
    for ci, (wx, wy) in enumerate(comps):
        for pos in range(3):
            c0 = pos_chunk[pos]
            start = min(max(R * c0 - 2, 0), H - KROWS)
            Wm = wst[ci * 3 + pos]
            for o in range(8):
                for r in range(R):
                    y = R * c0 + r
                    for k in range(-2, 3):
                        v = y + k
                        if v < 0:
                            v = -v
                        if v > H - 1:
                            v = 2 * (H - 1) - v
                        rp = v - start
                        assert 0 <= rp < KROWS, (c0, r, k, v, start)
                        for cc in range(3):
                            Wm[SLOT_BASE[cc] + rp, R * o + r] += wx * Ax[o, cc] * VX[k + 2]
                            Wm[SLOT_BASE[3 + cc] + rp, R * o + r] += wy * Ay[o, cc] * VY[k + 2]

    # pass-2: channel contractions.  moving tensors T in {g, gA, gB, gAB}
    wp = (w_merge[0].astype(np.float64)) * 0.5  # (8,) w' = w_merge/2
    lam = {
        0: np.array([0.0, 0.0, 1.0, 0.0]),   # g   -> C2
        1: np.array([0.0, 1.0, -1.0, 0.0]),  # gA  -> C1 - C2
        2: np.array([0.0, 0.0, -1.0, 1.0]),  # gB  -> C3 - C2
        3: np.array([1.0, -1.0, 1.0, -1.0]),  # gAB -> C0 - C1 + C2 - C3
    }
    wredc = np.zeros((4, 128, 64), np.float64)
    for T in range(4):
        for o in range(8):
            for r in range(R):
                for k in range(4):
                    wredc[T, R * o + r, R * k + r] = wp[o] * lam[T][k]
    wreds = np.zeros((128, 16), np.float64)
    for o in range(8):
        for r in range(R):
            wreds[R * o + r, r] = 1.0

    # partition shift matrices: out = lhsT.T @ in
    # wshift[0]=DS: out[p]=in[p-1]; wshift[1]=DN: out[p]=in[p+1]
    wshift = np.zeros((2, 128, 128), np.float64)
    for m in range(1, 128):
        wshift[0, m - 1, m] = 1.0
    for m in range(127):
        wshift[1, m + 1, m] = 1.0

    lt = low_t.reshape(-1).astype(np.float64)   # (8,)
    ht = high_t.reshape(-1).astype(np.float64)
    l2v = np.zeros((128, 1), np.float32)
    h2v = np.zeros((128, 1), np.float32)
    for o in range(8):
        l2v[R * o:R * (o + 1), 0] = (lt[o] * lt[o]) if lt[o] >= 0 else NEG_BIG
        h2v[R * o:R * (o + 1), 0] = (ht[o] * ht[o]) if ht[o] >= 0 else NEG_BIG

    return {
        "wst": wst.astype(ml_dtypes.bfloat16),
        "wredc": wredc.astype(ml_dtypes.bfloat16),
        "wreds": wreds.astype(ml_dtypes.bfloat16),
        "wshift": wshift.astype(ml_dtypes.bfloat16),
        "l2v": l2v,
        "h2v": h2v,
        "gauss_e": np.float32(e),
    }


def chunk_start(c, H):
    return min(max(R * c - 2, 0), H - KROWS)


def chunk_pos(c, H):
    CH = H // R
    return 0 if c == 0 else (2 if c == CH - 1 else 1)


def build_program(H, W, n_cores):
    """Emit the single-core Tile program (SPMD across cores)."""
    Wp = W + 4
    CH = H // R
    NS = H // SROWS
    PB = min(H, 128)           # packed-plane partitions
    HB = H // PB               # packed-plane row blocks
    E_GAUSS = math.exp(-1.0 / 18.0)
    assert H % SROWS == 0 and W % 2 == 0

    nc = bacc.Bacc("TRN2", target_bir_lowering=False, debug=False,
                   enable_asserts=True, num_devices=n_cores)

    xin_t = nc.dram_tensor("xin", (3, H, Wp), BF16, kind="ExternalInput")
    wst_t = nc.dram_tensor("wst", (9, NPART, 128), BF16, kind="ExternalInput")
    wredc_t = nc.dram_tensor("wredc", (4, 128, 64), BF16, kind="ExternalInput")
    wreds_t = nc.dram_tensor("wreds", (128, 16), BF16, kind="ExternalInput")
    wshift_t = nc.dram_tensor("wshift", (2, 128, 128), BF16, kind="ExternalInput")
    l2v_t = nc.dram_tensor("l2v", (128, 1), F32, kind="ExternalInput")
    h2v_t = nc.dram_tensor("h2v", (128, 1), F32, kind="ExternalInput")
    zer_t = nc.dram_tensor("zer", (4, SCH * W), BF16, kind="ExternalInput")
    out_t = nc.dram_tensor("out", (H, W), F32, kind="ExternalOutput")

    def dram_ap(t, offset, pairs):
        return bass.AP(t, offset, [list(p) for p in pairs])

    from contextlib import ExitStack
    with tile.TileContext(nc) as tc, ExitStack() as stack:
        cpool = stack.enter_context(tc.tile_pool(name="consts", bufs=1))
        wst = [cpool.tile([NPART, 128], BF16, tag=f"wst{i}", name=f"wst{i}") for i in range(9)]
        for i in range(9):
            nc.sync.dma_start(wst[i][:], wst_t.ap()[i])
        wredc = [cpool.tile([128, 64], BF16, tag=f"wredc{i}", name=f"wredc{i}") for i in range(4)]
        for i in range(4):
            nc.sync.dma_start(wredc[i][:], wredc_t.ap()[i])
        wreds = cpool.tile([128, 16], BF16, tag="wreds", name="wreds")
        nc.sync.dma_start(wreds[:], wreds_t.ap())
        wsh = [cpool.tile([128, 128], BF16, tag=f"wsh{i}", name=f"wsh{i}") for i in range(2)]
        for i in range(2):
            nc.sync.dma_start(wsh[i][:], wshift_t.ap()[i])
        l2v = cpool.tile([128, 1], F32, tag="l2v", name="l2v")
        nc.sync.dma_start(l2v[:], l2v_t.ap())
        h2v = cpool.tile([128, 1], F32, tag="h2v", name="h2v")
        nc.sync.dma_start(h2v[:], h2v_t.ap())
        epsv = cpool.tile([128, 1], F32, tag="epsv", name="epsv")
        nc.gpsimd.memset(epsv[:], float(EPS))

        # full-image planes (bf16): Cpl aligned [PB,HB,W]; spl padded cols
        ppool = stack.enter_context(tc.tile_pool(name="planes", bufs=1))
        Cpl = [ppool.tile([PB, HB, W], BF16, tag=f"C{k}", name=f"C{k}") for k in range(4)]
        spl = ppool.tile([PB, HB, W + 2], BF16, tag="spl", name="spl")
        nc.gpsimd.memset(spl[:, :, 0:1], 0.0)
        nc.gpsimd.memset(spl[:, :, W + 1:W + 2], 0.0)

        with (
            tc.tile_pool(name="xs", bufs=2) as xpool,
            tc.tile_pool(name="hz", bufs=2) as hpool,
            tc.tile_pool(name="sq", bufs=2) as spool,
            tc.tile_pool(name="dv", bufs=2) as dpool,
            tc.tile_pool(name="ps", bufs=4, space="PSUM") as mm1pool,
            tc.tile_pool(name="ps2", bufs=1, space="PSUM") as mm2pool,
        ):
            for s in range(NS):
                cbase = s * SCH
                # ---- load x (bf16 from host; xc2 = 1-col-shifted second load
                # so the center tap reads stay 4B-aligned on the DVE) ----
                xc = xpool.tile([60, SCH, Wp], BF16, tag="xc", name="xc")
                xc2 = xpool.tile([60, SCH, Wp - 2], BF16, tag="xc2", name="xc2")
                for slot in range(3):
                    ch = slot
                    runs = []
                    cur = []
                    for cl in range(SCH):
                        c = cbase + cl
                        st = chunk_start(c, H)
                        if cur and st != chunk_start(cbase + cur[-1], H) + R:
                            runs.append(cur)
                            cur = []
                        cur.append(cl)
                    runs.append(cur)
                    for run in runs:
                        st0 = chunk_start(cbase + run[0], H)
                        base = ch * H * Wp + st0 * Wp
                        nc.sync.dma_start(
                            xc[SLOT_BASE[slot]:SLOT_BASE[slot] + KROWS,
                               run[0]:run[0] + len(run), :],
                            dram_ap(xin_t, base,
                                    [(Wp, KROWS), (R * Wp, len(run)), (1, Wp)]))
                        nc.sync.dma_start(
                            xc2[SLOT_BASE[slot]:SLOT_BASE[slot] + KROWS,
                                run[0]:run[0] + len(run), :],
                            dram_ap(xin_t, base + 1,
                                    [(Wp, KROWS), (R * Wp, len(run)), (1, Wp - 2)]))

                # ---- horizontal FIRs (all aligned bf16, 2x/4x modes) ----
                t1 = hpool.tile([60, SCH, Wp - 2], BF16, tag="t1", name="t1")
                nc.vector.tensor_tensor(t1[:], xc[:, :, 0:Wp - 2],
                                        xc[:, :, 2:Wp], OP.add)
                ts1 = hpool.tile([60, SCH, Wp - 2], BF16, tag="ts1", name="ts1")
                nc.vector.tensor_scalar(ts1[:], t1[:], float(E_GAUSS), None, OP.mult)
                xb = hpool.tile([NPART, SCH, Wp - 2], BF16, tag="xb", name="xb")
                nc.vector.tensor_tensor(xb[0:60], xc2[:], ts1[:], OP.add)
                # duplicate gauss-H rows into the qy slots (partition move;
                # DMA is exempt from the engine partition-start rule)
                nc.sync.dma_start(xb[64:124], xb[0:60])

                Q = hpool.tile([NPART, SCH, W], BF16, tag="Q", name="Q")
                # zero the matmul pad rows 60..63
                nc.gpsimd.dma_start(Q[60:64], zer_t.ap().rearrange(
                    "p (a b) -> p a b", a=SCH))
                # qx = xb[w+1] - xb[w-1]   (slots 0..2)
                nc.vector.tensor_tensor(
                    Q[0:60], xb[0:60, :, 2:W + 2], xb[0:60, :, 0:W], OP.subtract)
                # qy = 2*xb[w] + (xb[w-1]+xb[w+1])   (slots 3..5)
                t2 = hpool.tile([NPART, SCH, W], BF16, tag="t2", name="t2")
                nc.vector.tensor_tensor(
                    t2[64:124], xb[64:124, :, 0:W], xb[64:124, :, 2:W + 2], OP.add)
                ts2 = hpool.tile([NPART, SCH, W], BF16, tag="ts2", name="ts2")
                nc.vector.tensor_scalar(ts2[64:124], xb[64:124, :, 1:W + 1], 2.0,
                                        None, OP.mult)
                nc.vector.tensor_tensor(Q[64:124], ts2[64:124], t2[64:124], OP.add)

                # ---- PE pass 1: vertical FIR + mix + rotation; squares out ----
                SP = [spool.tile([128, SCH, W], BF16, tag=f"sp{i}", name=f"sp{i}") for i in range(3)]
                for ci in range(3):
                    for cl in range(SCH):
                        c = cbase + cl
                        pm = mm1pool.tile([128, W], F32, tag="mm1", name="mm1")
                        nc.tensor.matmul(pm[:], wst[ci * 3 + chunk_pos(c, H)][:],
                                         Q[:, cl, :], start=True, stop=True)
                        nc.scalar.activation(SP[ci][:, cl, :], pm[:], AF.Square)

                # ---- DVE chain (bf16, [128, SCH*W]) ----
                sp1, sp2, sq1 = SP
                mag2 = dpool.tile([128, SCH, W], BF16, tag="mag2", name="mag2", bufs=1)
                nc.vector.tensor_tensor(mag2[:], sp1[:], sp2[:], OP.add)
                # beta = [sp1 >= sp2]
                bm = dpool.tile([128, SCH, W], BF16, tag="bm", name="bm", bufs=1)
                nc.vector.tensor_tensor(bm[:], sp1[:], sp2[:], OP.is_ge)
                # alpha = [sq1 >= mag2/2]
                h05 = dpool.tile([128, SCH, W], BF16, tag="h05", name="h05", bufs=1)
                nc.vector.tensor_scalar(h05[:], mag2[:], 0.5, None, OP.mult)
                am = dpool.tile([128, SCH, W], BF16, tag="am", name="am", bufs=1)
                nc.vector.tensor_tensor(am[:], sq1[:], h05[:], OP.is_ge)
                mag = dpool.tile([128, SCH, W], BF16, tag="mag", name="mag")
                nc.scalar.activation(mag[:], mag2[:], AF.Sqrt, bias=epsv[:])
                stepl = dpool.tile([128, SCH, W], BF16, tag="stepl", name="stepl", bufs=1)
                nc.vector.tensor_scalar(stepl[:], mag2[:], l2v[:], None, OP.is_ge)
                steph = dpool.tile([128, SCH, W], BF16, tag="steph", name="steph", bufs=1)
                nc.vector.tensor_scalar(steph[:], mag2[:], h2v[:], None, OP.is_ge)
                g = dpool.tile([128, SCH, W], BF16, tag="g", name="g")
                nc.vector.tensor_tensor(g[:], stepl[:], steph[:], OP.add)
                gA = dpool.tile([128, SCH, W], BF16, tag="gA", name="gA")
                nc.vector.tensor_tensor(gA[:], am[:], g[:], OP.mult)
                gB = dpool.tile([128, SCH, W], BF16, tag="gB", name="gB")
                nc.vector.tensor_tensor(gB[:], bm[:], g[:], OP.mult)
                gAB = dpool.tile([128, SCH, W], BF16, tag="gAB", name="gAB")
                nc.vector.tensor_tensor(gAB[:], am[:], gB[:], OP.mult)

                # ---- PE pass 2: channel contractions ----
                gT = [g, gA, gB, gAB]
                for half in range(2):
                    pc = mm2pool.tile([64, 2, W], F32, tag="mm2c", name="mm2c")
                    psm = mm2pool.tile([16, 2, W], F32, tag="mm2s", name="mm2s")
                    for cl2 in range(2):
                        cl = half * 2 + cl2
                        for T in range(4):
                            nc.tensor.matmul(pc[:, cl2, :], wredc[T][:], gT[T][:, cl, :],
                                             start=(T == 0), stop=(T == 3))
                        nc.tensor.matmul(psm[:, cl2, :], wreds[:], mag[:, cl, :],
                                         start=True, stop=True)
                    stC = spool.tile([64, 2, W], BF16, tag="stC", name="stC")
                    nc.scalar.activation(stC[:], pc[:], AF.Copy)
                    stS = spool.tile([16, 2, W], BF16, tag="stS", name="stS")
                    nc.scalar.activation(stS[:], psm[:], AF.Copy)
                    for cl2 in range(2):
                        c = cbase + half * 2 + cl2
                        pp = (c * R) % PB
                        blk = (c * R) // PB
                        for k in range(4):
                            nc.gpsimd.dma_start(Cpl[k][pp:pp + R, blk, :],
                                                stC[R * k:R * (k + 1), cl2, :])
                        nc.gpsimd.dma_start(spl[pp:pp + R, blk, 1:W + 1],
                                            stS[:, cl2, :])

        # ================= phase C: 1-channel tail =================
        with (
            tc.tile_pool(name="tail", bufs=1) as tp,
            tc.tile_pool(name="tps", bufs=8, space="PSUM") as tpp,
        ):
            # aligned center copy of s
            scen = tp.tile([PB, HB, W], BF16, tag="scen", name="scen")
            nc.vector.tensor_copy(scen[:], spl[:, :, 1:W + 1])

            # row-shifted s planes via PE shift matmuls: padded + centered evacs
            sSp = tp.tile([PB, HB, W + 2], BF16, tag="sSp", name="sSp")
            sNp = tp.tile([PB, HB, W + 2], BF16, tag="sNp", name="sNp")
            sSc = tp.tile([PB, HB, W], BF16, tag="sSc", name="sSc")
            sNc = tp.tile([PB, HB, W], BF16, tag="sNc", name="sNc")
            for t in (sSp, sNp):
                nc.gpsimd.memset(t[:, :, 0:1], 0.0)
                nc.gpsimd.memset(t[:, :, W + 1:W + 2], 0.0)
            for b in range(HB):
                pss = tpp.tile([128, W], F32, tag="pss", name=f"pss{b}", bufs=2)
                nc.tensor.matmul(pss[:], wsh[0][:], spl[:, b, 1:W + 1],
                                 start=True, stop=True)
                nc.scalar.activation(sSp[:, b, 1:W + 1], pss[:], AF.Copy)
                nc.scalar.activation(sSc[:, b, :], pss[:], AF.Copy)
                psn = tpp.tile([128, W], F32, tag="psn", name=f"psn{b}", bufs=2)
                nc.tensor.matmul(psn[:], wsh[1][:], spl[:, b, 1:W + 1],
                                 start=True, stop=True)
                nc.scalar.activation(sNp[:, b, 1:W + 1], psn[:], AF.Copy)
                nc.scalar.activation(sNc[:, b, :], psn[:], AF.Copy)
            # cross-block fixup rows (partition 0/127 wrap)
            if HB > 1:
                nc.sync.dma_start(sSp[0:1, 1:HB, 1:W + 1], scen[PB - 1:PB, 0:HB - 1, :])
                nc.sync.dma_start(sSc[0:1, 1:HB, :], scen[PB - 1:PB, 0:HB - 1, :])
                nc.gpsimd.dma_start(sNp[PB - 1:PB, 0:HB - 1, 1:W + 1], scen[0:1, 1:HB, :])
                nc.gpsimd.dma_start(sNc[PB - 1:PB, 0:HB - 1, :], scen[0:1, 1:HB, :])
            # boundary rows (y=-1, y=H) are zero straight from the PE shift.

            # mx_k = max of the two opposite shifted neighbors; b_k = [s > mx_k]
            # pairs: (E,W), (SE,NW), (S,N), (SW,NE)
            shift_pairs = [
                ((spl, 2), (spl, 0)),
                ((sSp, 2), (sNp, 0)),
                ((sSc, None), (sNc, None)),
                ((sSp, 0), (sNp, 2)),
            ]
            # merge in f32: bf16 products are exact in f32, so the m==1.0 /
            # m==0.5 equality gates keep f32-level discrimination (a bf16
            # accumulation would false-fire ~2^9 times more often)
            acc = None
            for k, ((ta, oa), (tb, ob)) in enumerate(shift_pairs):
                va = ta[:] if oa is None else ta[:, :, oa:oa + W]
                vb = tb[:] if ob is None else tb[:, :, ob:ob + W]
                mx = tp.tile([PB, HB, W], BF16, tag="mx", name="mx", bufs=2)
                nc.vector.tensor_tensor(mx[:], va, vb, OP.max)
                bk = tp.tile([PB, HB, W], BF16, tag="bk", name="bk", bufs=2)
                nc.vector.tensor_tensor(bk[:], scen[:], mx[:], OP.is_gt)
                pk = tp.tile([PB, HB, W], F32, tag="pk", name="pk", bufs=2)
                nc.vector.tensor_tensor(pk[:], bk[:], Cpl[k][:], OP.mult)
                if acc is None:
                    acc = pk
                else:
                    acc2 = tp.tile([PB, HB, W], F32, tag="macc", name="macc", bufs=2)
                    nc.vector.tensor_tensor(acc2[:], acc[:], pk[:], OP.add)
                    acc = acc2
            m = acc  # merged map, f32

            # hysteresis: the strong map iS = (m==1.0)*m equals e1 exactly,
            # and all stencil sums are small integers — exact in bf16
            e1 = tp.tile([PB, HB, W], BF16, tag="e1", name="e1")
            nc.vector.tensor_scalar(e1[:], m[:], 1.0, None, OP.is_equal)
            iSc = e1
            # column-padded copy (reflect pad) for the l/r taps
            iSp = tp.tile([PB, HB, W + 2], BF16, tag="iSp", name="iSp")
            nc.vector.tensor_copy(iSp[:, :, 1:W + 1], iSc[:])
            nc.vector.tensor_copy(iSp[:, :, 0:1], iSc[:, :, 1:2])
            nc.vector.tensor_copy(iSp[:, :, W + 1:W + 2], iSc[:, :, W - 2:W - 1])
            th = tp.tile([PB, HB, W], BF16, tag="th", name="th")
            nc.vector.tensor_tensor(th[:], iSp[:, :, 0:W], iSp[:, :, 2:W + 2], OP.add)
            hh = tp.tile([PB, HB, W], BF16, tag="hh", name="hh")
            nc.vector.tensor_tensor(hh[:], th[:], iSc[:], OP.add)
            # row shifts of hh via PE (reflect rows at image boundary)
            hhN = tp.tile([PB, HB, W], BF16, tag="hhN", name="hhN")
            hhS = tp.tile([PB, HB, W], BF16, tag="hhS", name="hhS")
            for b in range(HB):
                ph1 = tpp.tile([128, W], F32, tag="ph1", name=f"ph1{b}", bufs=2)
                nc.tensor.matmul(ph1[:], wsh[1][:], hh[:, b, :], start=True, stop=True)
                nc.scalar.activation(hhN[:, b, :], ph1[:], AF.Copy)
                ph2 = tpp.tile([128, W], F32, tag="ph2", name=f"ph2{b}", bufs=2)
                nc.tensor.matmul(ph2[:], wsh[0][:], hh[:, b, :], start=True, stop=True)
                nc.scalar.activation(hhS[:, b, :], ph2[:], AF.Copy)
            if HB > 1:
                nc.sync.dma_start(hhN[PB - 1:PB, 0:HB - 1, :], hh[0:1, 1:HB, :])
                nc.gpsimd.dma_start(hhS[0:1, 1:HB, :], hh[PB - 1:PB, 0:HB - 1, :])
            # reflect at y=H-1 (row H-2) and y=0 (row 1)
            nc.scalar.dma_start(hhN[PB - 1:PB, HB - 1:HB, :], hh[PB - 2:PB - 1, HB - 1:HB, :])
            nc.scalar.dma_start(hhS[0:1, 0:1, :], hh[1:2, 0:1, :])

            vv = tp.tile([PB, HB, W], BF16, tag="vv", name="vv")
            nc.vector.tensor_tensor(vv[:], hhN[:], hhS[:], OP.add)
            hv = tp.tile([PB, HB, W], BF16, tag="hv", name="hv")
            nc.vector.tensor_tensor(hv[:], vv[:], hh[:], OP.add)
            # hyst = 1.25*hv > 1  <=>  hv > 0.8 (hv is a small integer count)
            hgt = tp.tile([PB, HB, W], BF16, tag="hgt", name="hgt")
            nc.vector.tensor_scalar(hgt[:], hv[:], 0.8, None, OP.is_gt)
            m5 = tp.tile([PB, HB, W], BF16, tag="m5", name="m5")
            nc.vector.tensor_scalar(m5[:], m[:], 0.5, None, OP.is_equal)
            w1 = tp.tile([PB, HB, W], BF16, tag="w1", name="w1")
            nc.vector.tensor_tensor(w1[:], hgt[:], m5[:], OP.mult)
            # w2 = w1*m, but w1 is only nonzero where m==0.5 exactly
            w2 = tp.tile([PB, HB, W], BF16, tag="w2", name="w2")
            nc.vector.tensor_scalar(w2[:], w1[:], 0.5, None, OP.mult)
            outb = tp.tile([PB, HB, W], BF16, tag="outb", name="outb")
            nc.vector.tensor_tensor(outb[:], w2[:], iSc[:], OP.add)
            outv = tp.tile([PB, HB, W], F32, tag="outv", name="outv")
            nc.scalar.activation(outv[:], outb[:], AF.Copy)
            nc.sync.dma_start(
                dram_ap(out_t, 0, [(W, PB), (PB * W, HB), (1, W)]), outv[:])

    nc.compile()
    return nc


_PROG_CACHE = {}


def _get_program(H, W, n_cores):
    key = (H, W, n_cores)
    if key not in _PROG_CACHE:
        _PROG_CACHE[key] = build_program(H, W, n_cores)
    return _PROG_CACHE[key]


def make_in_maps(x, w_expand, w_perm_gauss, w_perm_sx, w_perm_sy, w_merge,
                 low_t, high_t):
    B, Cin, H, W = x.shape
    wd = make_weights(np.asarray(w_expand, np.float64), np.asarray(w_perm_gauss, np.float64),
                      np.asarray(w_perm_sx, np.float64), np.asarray(w_perm_sy, np.float64),
                      np.asarray(w_merge, np.float64), np.asarray(low_t), np.asarray(high_t), H)
    xpad = np.pad(np.asarray(x, np.float32), ((0, 0), (0, 0), (0, 0), (2, 2)),
                  mode="reflect").astype(ml_dtypes.bfloat16)
    shared = {
        "wst": np.ascontiguousarray(wd["wst"]),
        "wredc": np.ascontiguousarray(wd["wredc"]),
        "wreds": np.ascontiguousarray(wd["wreds"]),
        "wshift": np.ascontiguousarray(wd["wshift"]),
        "l2v": wd["l2v"], "h2v": wd["h2v"],
        "zer": np.zeros((4, SCH * W), ml_dtypes.bfloat16),
    }
    return [dict(shared, xin=np.ascontiguousarray(xpad[b])) for b in range(B)]


def kernel(x, w_expand, w_perm_gauss, w_perm_sx, w_perm_sy, w_merge,
           low_t, high_t):
    x = np.asarray(x)
    B, Cin, H, W = x.shape
    assert (B, Cin) == (8, 3)
    nc = _get_program(H, W, 8)
    in_maps = make_in_maps(x, w_expand, w_perm_gauss, w_perm_sx, w_perm_sy,
                           w_merge, low_t, high_t)
    res = run_bass_kernel_spmd(nc, in_maps, core_ids=list(range(8)))
    out = np.stack([res.results[b]["out"] for b in range(8)])[:, None]
    return out.astype(np.float32)


# revision 15
# speedup vs baseline: 1.0153x; 1.0153x over previous
"""Trainium2 Bass kernel for nn_CannyDetector (B=8, Cin=3, C=8, H=W=512).

Strategy (pure data parallel, one image per NeuronCore, 8 cores):

Pipeline per core (see reference):
    h  = Wexp @ x; g = Pg @ gauss3x3(h); gx = Px @ sobelx(g); gy = Py @ sobely(g)
    mag = sqrt(gx^2+gy^2+eps); s = sum_c mag; dirs_k = s - shift_k(s)
    idx = round(atan2(gy,gx)/45deg); nms = mag * (dirs[idx]>0 & dirs[idx+4]>0)
    thr = step functions of mag; m = Wmerge @ thr; out = hysteresis(m)

All depthwise stencils and 1x1 channel mixes are linear, regrouped as:
3-tap horizontal FIRs on the 3 input channels (VectorE, all ops in
aligned bf16 2x/4x perf modes; the shared gauss-H row is computed once
and duplicated across partition slots by a small SBUF->SBUF DMA), then a
TensorE pass applying 5-tap vertical FIRs + 3->8 channel mixes +
a +22.5deg rotation, emitting p1,p2,q1=(p1+p2)/sqrt2 per channel.
Squares (ScalarE) give mag2 = p1^2+p2^2 and NMS sector bits as sign
tests beta=[p1^2>=p2^2], alpha=[2*q1^2>=mag2] (bit-exact vs
round(atan2)%4). The double threshold reduces to step functions of mag2.
A second TensorE pass contracts channels with merge weights into 4
axis-plane C_k and s.  The 1-channel tail runs on packed [128, H/128, W]
planes: row+-1 shifted planes come from TensorE shift-matrix matmuls
(not bulk DMA), column shifts from free-dim offsets of padded tiles,
so the tail's critical path is ~20 cheap bf16 VectorE ops.
"""

import math
import sys

import numpy as np

if "/opt/trn_rl_repo" not in sys.path:
    sys.path.insert(0, "/opt/trn_rl_repo")

import ml_dtypes

import concourse.bass as bass
import concourse.bacc as bacc
import concourse.tile as tile
from concourse import mybir
from concourse.bass_utils import run_bass_kernel_spmd

F32 = mybir.dt.float32
BF16 = mybir.dt.bfloat16
OP = mybir.AluOpType
AF = mybir.ActivationFunctionType

EPS = 1e-10
NEG_BIG = -1.0e30

R = 16          # chunk rows (PE pass granularity)
KROWS = R + 4   # moving-operand rows per chunk (5-tap vertical halo)
SCH = 4         # chunks per stripe
SROWS = R * SCH  # rows per stripe
SLOT_BASE = [0, 20, 40, 64, 84, 104]   # partition base per (qx0..2, qy0..2)
NPART = 124     # moving-operand partitions (rows 60..63 are zero padding)


def make_weights(w_expand, w_perm_gauss, w_perm_sx, w_perm_sy, w_merge,
                 low_t, high_t, H):
    """Host-side prep of stationary matrices and per-partition vectors."""
    e = math.exp(-1.0 / 18.0)
    norm = 1.0 / (2.0 * e + 1.0)
    # DVE computes xb = center + e*(left+right) = (2e+1)*gaussH; fold norm here
    phi_h = norm

    Ax = (w_perm_sx @ w_perm_gauss @ w_expand).astype(np.float64)   # (8,3)
    Ay = (w_perm_sy @ w_perm_gauss @ w_expand).astype(np.float64)

    gv = np.array([e, 1.0, e]) * norm            # vertical gauss (offsets -1..1)
    svx = np.array([0.5, 1.0, 0.5])              # sobel-x vertical
    svy = np.array([-1.0, 0.0, 1.0])             # sobel-y vertical
    VX = np.convolve(svx, gv) * phi_h            # offsets -2..2
    VY = np.convolve(svy, gv) * phi_h * 0.5      # qy carries 2x, fold 0.5

    cth, sth = math.cos(math.pi / 8), math.sin(math.pi / 8)
    rt = math.sqrt(0.5)
    comps = [
        (cth, -sth),                  # p1
        (sth, cth),                   # p2
        ((cth + sth) * rt, (cth - sth) * rt),  # q1 = (p1+p2)/sqrt2
    ]

    CH = H // R
    # stationaries [9, NPART, 128]: comp-major, pos in {top(c=0), mid, bot}
    wst = np.zeros((9, NPART, 128), np.float64)
    pos_chunk = {0: 0, 1: min(1, CH - 1), 2: CH - 1}
    for ci, (wx, wy) in enumerate(comps):
        for pos in range(3):
            c0 = pos_chunk[pos]
            start = min(max(R * c0 - 2, 0), H - KROWS)
            Wm = wst[ci * 3 + pos]
            for o in range(8):
                for r in range(R):
                    y = R * c0 + r
                    for k in range(-2, 3):
                        v = y + k
                        if v < 0:
                            v = -v
                        if v > H - 1:
                            v = 2 * (H - 1) - v
                        rp = v - start
                        assert 0 <= rp < KROWS, (c0, r, k, v, start)
                        for cc in range(3):
                            Wm[SLOT_BASE[cc] + rp, R * o + r] += wx * Ax[o, cc] * VX[k + 2]
                            Wm[SLOT_BASE[3 + cc] + rp, R * o + r] += wy * Ay[o, cc] * VY[k + 2]

    # pass-2: channel contractions.  moving tensors T in {g, gA, gB, gAB}
    wp = (w_merge[0].astype(np.float64)) * 0.5  # (8,) w' = w_merge/2
    lam = {
        0: np.array([0.0, 0.0, 1.0, 0.0]),   # g   -> C2
        1: np.array([0.0, 1.0, -1.0, 0.0]),  # gA  -> C1 - C2
        2: np.array([0.0, 0.0, -1.0, 1.0]),  # gB  -> C3 - C2
        3: np.array([1.0, -1.0, 1.0, -1.0]),  # gAB -> C0 - C1 + C2 - C3
    }
    wredc = np.zeros((4, 128, 64), np.float64)
    for T in range(4):
        for o in range(8):
            for r in range(R):
                for k in range(4):
                    wredc[T, R * o + r, R * k + r] = wp[o] * lam[T][k]
    wreds = np.zeros((128, 16), np.float64)
    for o in range(8):
        for r in range(R):
            wreds[R * o + r, r] = 1.0

    # partition shift matrices: out = lhsT.T @ in
    # wshift[0]=DS: out[p]=in[p-1]; wshift[1]=DN: out[p]=in[p+1]
    wshift = np.zeros((2, 128, 128), np.float64)
    for m in range(1, 128):
        wshift[0, m - 1, m] = 1.0
    for m in range(127):
        wshift[1, m + 1, m] = 1.0

    lt = low_t.reshape(-1).astype(np.float64)   # (8,)
    ht = high_t.reshape(-1).astype(np.float64)
    l2v = np.zeros((128, 1), np.float32)
    h2v = np.zeros((128, 1), np.float32)
    for o in range(8):
        l2v[R * o:R * (o + 1), 0] = (lt[o] * lt[o]) if lt[o] >= 0 else NEG_BIG
        h2v[R * o:R * (o + 1), 0] = (ht[o] * ht[o]) if ht[o] >= 0 else NEG_BIG

    return {
        "wst": wst.astype(ml_dtypes.bfloat16),
        "wredc": wredc.astype(ml_dtypes.bfloat16),
        "wreds": wreds.astype(ml_dtypes.bfloat16),
        "wshift": wshift.astype(ml_dtypes.bfloat16),
        "l2v": l2v,
        "h2v": h2v,
        "gauss_e": np.float32(e),
    }


def chunk_start(c, H):
    return min(max(R * c - 2, 0), H - KROWS)


def chunk_pos(c, H):
    CH = H // R
    return 0 if c == 0 else (2 if c == CH - 1 else 1)


def build_program(H, W, n_cores):
    """Emit the single-core Tile program (SPMD across cores)."""
    Wp = W + 4
    CH = H // R
    NS = H // SROWS
    PB = min(H, 128)           # packed-plane partitions
    HB = H // PB               # packed-plane row blocks
    E_GAUSS = math.exp(-1.0 / 18.0)
    assert H % SROWS == 0 and W % 2 == 0

    nc = bacc.Bacc("TRN2", target_bir_lowering=False, debug=False,
                   enable_asserts=True, num_devices=n_cores)

    xin_t = nc.dram_tensor("xin", (3, H, Wp), BF16, kind="ExternalInput")
    wst_t = nc.dram_tensor("wst", (9, NPART, 128), BF16, kind="ExternalInput")
    wredc_t = nc.dram_tensor("wredc", (4, 128, 64), BF16, kind="ExternalInput")
    wreds_t = nc.dram_tensor("wreds", (128, 16), BF16, kind="ExternalInput")
    wshift_t = nc.dram_tensor("wshift", (2, 128, 128), BF16, kind="ExternalInput")
    l2v_t = nc.dram_tensor("l2v", (128, 1), F32, kind="ExternalInput")
    h2v_t = nc.dram_tensor("h2v", (128, 1), F32, kind="ExternalInput")
    zer_t = nc.dram_tensor("zer", (4, SCH * W), BF16, kind="ExternalInput")
    out_t = nc.dram_tensor("out", (H, W), F32, kind="ExternalOutput")

    def dram_ap(t, offset, pairs):
        return bass.AP(t, offset, [list(p) for p in pairs])

    from contextlib import ExitStack
    with tile.TileContext(nc) as tc, ExitStack() as stack:
        cpool = stack.enter_context(tc.tile_pool(name="consts", bufs=1))
        wst = [cpool.tile([NPART, 128], BF16, tag=f"wst{i}", name=f"wst{i}") for i in range(9)]
        for i in range(9):
            nc.sync.dma_start(wst[i][:], wst_t.ap()[i])
        wredc = [cpool.tile([128, 64], BF16, tag=f"wredc{i}", name=f"wredc{i}") for i in range(4)]
        for i in range(4):
            nc.sync.dma_start(wredc[i][:], wredc_t.ap()[i])
        wreds = cpool.tile([128, 16], BF16, tag="wreds", name="wreds")
        nc.sync.dma_start(wreds[:], wreds_t.ap())
        wsh = [cpool.tile([128, 128], BF16, tag=f"wsh{i}", name=f"wsh{i}") for i in range(2)]
        for i in range(2):
            nc.sync.dma_start(wsh[i][:], wshift_t.ap()[i])
        l2v = cpool.tile([128, 1], F32, tag="l2v", name="l2v")
        nc.sync.dma_start(l2v[:], l2v_t.ap())
        h2v = cpool.tile([128, 1], F32, tag="h2v", name="h2v")
        nc.sync.dma_start(h2v[:], h2v_t.ap())
        epsv = cpool.tile([128, 1], F32, tag="epsv", name="epsv")
        nc.gpsimd.memset(epsv[:], float(EPS))

        # full-image planes (bf16): Cpl aligned [PB,HB,W]; spl padded cols
        ppool = stack.enter_context(tc.tile_pool(name="planes", bufs=1))
        Cpl = [ppool.tile([PB, HB, W], BF16, tag=f"C{k}", name=f"C{k}") for k in range(4)]
        spl = ppool.tile([PB, HB, W + 2], BF16, tag="spl", name="spl")
        nc.gpsimd.memset(spl[:, :, 0:1], 0.0)
        nc.gpsimd.memset(spl[:, :, W + 1:W + 2], 0.0)

        with (
            tc.tile_pool(name="xs", bufs=2) as xpool,
            tc.tile_pool(name="hz", bufs=2) as hpool,
            tc.tile_pool(name="sq", bufs=2) as spool,
            tc.tile_pool(name="dv", bufs=2) as dpool,
            tc.tile_pool(name="ps", bufs=4, space="PSUM") as mm1pool,
            tc.tile_pool(name="ps2", bufs=1, space="PSUM") as mm2pool,
        ):
            for s in range(NS):
                cbase = s * SCH
                # ---- load x (bf16 from host; xc2 = 1-col-shifted second load
                # so the center tap reads stay 4B-aligned on the DVE).
                # One 60-partition DMA covers all 3 channel slots. ----
                xc = xpool.tile([60, SCH, Wp], BF16, tag="xc", name="xc", bufs=3)
                xc2 = xpool.tile([60, SCH, Wp - 2], BF16, tag="xc2", name="xc2", bufs=3)
                runs = []
                cur = []
                for cl in range(SCH):
                    c = cbase + cl
                    st = chunk_start(c, H)
                    if cur and st != chunk_start(cbase + cur[-1], H) + R:
                        runs.append(cur)
                        cur = []
                    cur.append(cl)
                runs.append(cur)
                for run in runs:
                    st0 = chunk_start(cbase + run[0], H)
                    for ch in range(3):
                        base = ch * H * Wp + st0 * Wp
                        sl = SLOT_BASE[ch]
                        nc.sync.dma_start(
                            xc[sl:sl + KROWS, run[0]:run[0] + len(run), :],
                            dram_ap(xin_t, base,
                                    [(Wp, KROWS), (R * Wp, len(run)), (1, Wp)]))
                        nc.scalar.dma_start(
                            xc2[sl:sl + KROWS, run[0]:run[0] + len(run), :],
                            dram_ap(xin_t, base + 1,
                                    [(Wp, KROWS), (R * Wp, len(run)), (1, Wp - 2)]))

                # ---- horizontal FIRs (all aligned bf16, 2x/4x modes) ----
                t1 = hpool.tile([60, SCH, Wp - 2], BF16, tag="t1", name="t1")
                nc.vector.tensor_tensor(t1[:], xc[:, :, 0:Wp - 2],
                                        xc[:, :, 2:Wp], OP.add)
                ts1 = hpool.tile([60, SCH, Wp - 2], BF16, tag="ts1", name="ts1")
                nc.vector.tensor_scalar(ts1[:], t1[:], float(E_GAUSS), None, OP.mult)
                xb = hpool.tile([NPART, SCH, Wp - 2], BF16, tag="xb", name="xb")
                nc.vector.tensor_tensor(xb[0:60], xc2[:], ts1[:], OP.add)
                # duplicate gauss-H rows into the qy slots (partition move;
                # DMA is exempt from the engine partition-start rule)
                nc.sync.dma_start(xb[64:124], xb[0:60])

                Q = hpool.tile([NPART, SCH, W], BF16, tag="Q", name="Q")
                # zero the matmul pad rows 60..63
                nc.gpsimd.dma_start(Q[60:64], zer_t.ap().rearrange(
                    "p (a b) -> p a b", a=SCH))
                # qx = xb[w+1] - xb[w-1]   (slots 0..2)
                nc.vector.tensor_tensor(
                    Q[0:60], xb[0:60, :, 2:W + 2], xb[0:60, :, 0:W], OP.subtract)
                # qy = 2*xb[w] + (xb[w-1]+xb[w+1])   (slots 3..5)
                t2 = hpool.tile([NPART, SCH, W], BF16, tag="t2", name="t2")
                nc.vector.tensor_tensor(
                    t2[64:124], xb[64:124, :, 0:W], xb[64:124, :, 2:W + 2], OP.add)
                ts2 = hpool.tile([NPART, SCH, W], BF16, tag="ts2", name="ts2")
                nc.vector.tensor_scalar(ts2[64:124], xb[64:124, :, 1:W + 1], 2.0,
                                        None, OP.mult)
                nc.vector.tensor_tensor(Q[64:124], ts2[64:124], t2[64:124], OP.add)

                # ---- PE pass 1: vertical FIR + mix + rotation; squares out ----
                SP = [spool.tile([128, SCH, W], BF16, tag=f"sp{i}", name=f"sp{i}") for i in range(3)]
                for ci in range(3):
                    for cl in range(SCH):
                        c = cbase + cl
                        pm = mm1pool.tile([128, W], F32, tag="mm1", name="mm1")
                        nc.tensor.matmul(pm[:], wst[ci * 3 + chunk_pos(c, H)][:],
                                         Q[:, cl, :], start=True, stop=True)
                        nc.scalar.activation(SP[ci][:, cl, :], pm[:], AF.Square)

                # ---- DVE chain (bf16, [128, SCH*W]) ----
                sp1, sp2, sq1 = SP
                mag2 = dpool.tile([128, SCH, W], BF16, tag="mag2", name="mag2", bufs=1)
                nc.vector.tensor_tensor(mag2[:], sp1[:], sp2[:], OP.add)
                # beta = [sp1 >= sp2]
                bm = dpool.tile([128, SCH, W], BF16, tag="bm", name="bm", bufs=1)
                nc.vector.tensor_tensor(bm[:], sp1[:], sp2[:], OP.is_ge)
                # alpha = [sq1 >= mag2/2]
                h05 = dpool.tile([128, SCH, W], BF16, tag="h05", name="h05", bufs=1)
                nc.vector.tensor_scalar(h05[:], mag2[:], 0.5, None, OP.mult)
                am = dpool.tile([128, SCH, W], BF16, tag="am", name="am", bufs=1)
                nc.vector.tensor_tensor(am[:], sq1[:], h05[:], OP.is_ge)
                mag = dpool.tile([128, SCH, W], BF16, tag="mag", name="mag")
                nc.scalar.activation(mag[:], mag2[:], AF.Sqrt, bias=epsv[:])
                stepl = dpool.tile([128, SCH, W], BF16, tag="stepl", name="stepl", bufs=1)
                nc.vector.tensor_scalar(stepl[:], mag2[:], l2v[:], None, OP.is_ge)
                steph = dpool.tile([128, SCH, W], BF16, tag="steph", name="steph", bufs=1)
                nc.vector.tensor_scalar(steph[:], mag2[:], h2v[:], None, OP.is_ge)
                g = dpool.tile([128, SCH, W], BF16, tag="g", name="g")
                nc.vector.tensor_tensor(g[:], stepl[:], steph[:], OP.add)
                gA = dpool.tile([128, SCH, W], BF16, tag="gA", name="gA")
                nc.vector.tensor_tensor(gA[:], am[:], g[:], OP.mult)
                gB = dpool.tile([128, SCH, W], BF16, tag="gB", name="gB")
                nc.vector.tensor_tensor(gB[:], bm[:], g[:], OP.mult)
                gAB = dpool.tile([128, SCH, W], BF16, tag="gAB", name="gAB")
                nc.vector.tensor_tensor(gAB[:], am[:], gB[:], OP.mult)

                # ---- PE pass 2: channel contractions ----
                gT = [g, gA, gB, gAB]
                for half in range(2):
                    pc = mm2pool.tile([64, 2, W], F32, tag="mm2c", name="mm2c")
                    psm = mm2pool.tile([16, 2, W], F32, tag="mm2s", name="mm2s")
                    for cl2 in range(2):
                        cl = half * 2 + cl2
                        for T in range(4):
                            nc.tensor.matmul(pc[:, cl2, :], wredc[T][:], gT[T][:, cl, :],
                                             start=(T == 0), stop=(T == 3))
                        nc.tensor.matmul(psm[:, cl2, :], wreds[:], mag[:, cl, :],
                                         start=True, stop=True)
                    stC = spool.tile([64, 2, W], BF16, tag="stC", name="stC")
                    nc.scalar.activation(stC[:], pc[:], AF.Copy)
                    stS = spool.tile([16, 2, W], BF16, tag="stS", name="stS")
                    nc.scalar.activation(stS[:], psm[:], AF.Copy)
                    for cl2 in range(2):
                        c = cbase + half * 2 + cl2
                        pp = (c * R) % PB
                        blk = (c * R) // PB
                        for k in range(4):
                            nc.gpsimd.dma_start(Cpl[k][pp:pp + R, blk, :],
                                                stC[R * k:R * (k + 1), cl2, :])
                        nc.gpsimd.dma_start(spl[pp:pp + R, blk, 1:W + 1],
                                            stS[:, cl2, :])

        # ================= phase C: 1-channel tail =================
        with (
            tc.tile_pool(name="tail", bufs=1) as tp,
            tc.tile_pool(name="tps", bufs=8, space="PSUM") as tpp,
        ):
            # aligned center copy of s
            scen = tp.tile([PB, HB, W], BF16, tag="scen", name="scen")
            nc.vector.tensor_copy(scen[:], spl[:, :, 1:W + 1])

            # row-shifted s planes via PE shift matmuls: padded + centered evacs
            sSp = tp.tile([PB, HB, W + 2], BF16, tag="sSp", name="sSp")
            sNp = tp.tile([PB, HB, W + 2], BF16, tag="sNp", name="sNp")
            sSc = tp.tile([PB, HB, W], BF16, tag="sSc", name="sSc")
            sNc = tp.tile([PB, HB, W], BF16, tag="sNc", name="sNc")
            for t in (sSp, sNp):
                nc.gpsimd.memset(t[:, :, 0:1], 0.0)
                nc.gpsimd.memset(t[:, :, W + 1:W + 2], 0.0)
            for b in range(HB):
                pss = tpp.tile([128, W], F32, tag="pss", name=f"pss{b}", bufs=2)
                nc.tensor.matmul(pss[:], wsh[0][:], spl[:, b, 1:W + 1],
                                 start=True, stop=True)
                nc.scalar.activation(sSp[:, b, 1:W + 1], pss[:], AF.Copy)
                nc.scalar.activation(sSc[:, b, :], pss[:], AF.Copy)
                psn = tpp.tile([128, W], F32, tag="psn", name=f"psn{b}", bufs=2)
                nc.tensor.matmul(psn[:], wsh[1][:], spl[:, b, 1:W + 1],
                                 start=True, stop=True)
                nc.scalar.activation(sNp[:, b, 1:W + 1], psn[:], AF.Copy)
                nc.scalar.activation(sNc[:, b, :], psn[:], AF.Copy)
            # cross-block fixup rows (partition 0/127 wrap)
            if HB > 1:
                nc.sync.dma_start(sSp[0:1, 1:HB, 1:W + 1], scen[PB - 1:PB, 0:HB - 1, :])
                nc.sync.dma_start(sSc[0:1, 1:HB, :], scen[PB - 1:PB, 0:HB - 1, :])
                nc.gpsimd.dma_start(sNp[PB - 1:PB, 0:HB - 1, 1:W + 1], scen[0:1, 1:HB, :])
                nc.gpsimd.dma_start(sNc[PB - 1:PB, 0:HB - 1, :], scen[0:1, 1:HB, :])
            # boundary rows (y=-1, y=H) are zero straight from the PE shift.

            # mx_k = max of the two opposite shifted neighbors; b_k = [s > mx_k]
            # pairs: (E,W), (SE,NW), (S,N), (SW,NE)
            shift_pairs = [
                ((spl, 2), (spl, 0)),
                ((sSp, 2), (sNp, 0)),
                ((sSc, None), (sNc, None)),
                ((sSp, 0), (sNp, 2)),
            ]
            # merge in f32: bf16 products are exact in f32, so the m==1.0 /
            # m==0.5 equality gates keep f32-level discrimination (a bf16
            # accumulation would false-fire ~2^9 times more often)
            acc = None
            for k, ((ta, oa), (tb, ob)) in enumerate(shift_pairs):
                va = ta[:] if oa is None else ta[:, :, oa:oa + W]
                vb = tb[:] if ob is None else tb[:, :, ob:ob + W]
                mx = tp.tile([PB, HB, W], BF16, tag="mx", name="mx", bufs=2)
                nc.vector.tensor_tensor(mx[:], va, vb, OP.max)
                bk = tp.tile([PB, HB, W], BF16, tag="bk", name="bk", bufs=2)
                nc.vector.tensor_tensor(bk[:], scen[:], mx[:], OP.is_gt)
                pk = tp.tile([PB, HB, W], F32, tag="pk", name="pk", bufs=2)
                nc.vector.tensor_tensor(pk[:], bk[:], Cpl[k][:], OP.mult)
                if acc is None:
                    acc = pk
                else:
                    acc2 = tp.tile([PB, HB, W], F32, tag="macc", name="macc", bufs=2)
                    nc.vector.tensor_tensor(acc2[:], acc[:], pk[:], OP.add)
                    acc = acc2
            m = acc  # merged map, f32

            # hysteresis: the strong map iS = (m==1.0)*m equals e1 exactly,
            # and all stencil sums are small integers — exact in bf16
            e1 = tp.tile([PB, HB, W], BF16, tag="e1", name="e1")
            nc.vector.tensor_scalar(e1[:], m[:], 1.0, None, OP.is_equal)
            iSc = e1
            # column-padded copy (reflect pad) for the l/r taps
            iSp = tp.tile([PB, HB, W + 2], BF16, tag="iSp", name="iSp")
            nc.vector.tensor_copy(iSp[:, :, 1:W + 1], iSc[:])
            nc.vector.tensor_copy(iSp[:, :, 0:1], iSc[:, :, 1:2])
            nc.vector.tensor_copy(iSp[:, :, W + 1:W + 2], iSc[:, :, W - 2:W - 1])
            th = tp.tile([PB, HB, W], BF16, tag="th", name="th")
            nc.vector.tensor_tensor(th[:], iSp[:, :, 0:W], iSp[:, :, 2:W + 2], OP.add)
            hh = tp.tile([PB, HB, W], BF16, tag="hh", name="hh")
            nc.vector.tensor_tensor(hh[:], th[:], iSc[:], OP.add)
            # row shifts of hh via PE (reflect rows at image boundary)
            hhN = tp.tile([PB, HB, W], BF16, tag="hhN", name="hhN")
            hhS = tp.tile([PB, HB, W], BF16, tag="hhS", name="hhS")
            for b in range(HB):
                ph1 = tpp.tile([128, W], F32, tag="ph1", name=f"ph1{b}", bufs=2)
                nc.tensor.matmul(ph1[:], wsh[1][:], hh[:, b, :], start=True, stop=True)
                nc.scalar.activation(hhN[:, b, :], ph1[:], AF.Copy)
                ph2 = tpp.tile([128, W], F32, tag="ph2", name=f"ph2{b}", bufs=2)
                nc.tensor.matmul(ph2[:], wsh[0][:], hh[:, b, :], start=True, stop=True)
                nc.scalar.activation(hhS[:, b, :], ph2[:], AF.Copy)
            if HB > 1:
                nc.sync.dma_start(hhN[PB - 1:PB, 0:HB - 1, :], hh[0:1, 1:HB, :])
                nc.gpsimd.dma_start(hhS[0:1, 1:HB, :], hh[PB - 1:PB, 0:HB - 1, :])
            # reflect at y=H-1 (row H-2) and y=0 (row 1)
            nc.scalar.dma_start(hhN[PB - 1:PB, HB - 1:HB, :], hh[PB - 2:PB - 1, HB - 1:HB, :])
            nc.scalar.dma_start(hhS[0:1, 0:1, :], hh[1:2, 0:1, :])

            vv = tp.tile([PB, HB, W], BF16, tag="vv", name="vv")
            nc.vector.tensor_tensor(vv[:], hhN[:], hhS[:], OP.add)
            hv = tp.tile([PB, HB, W], BF16, tag="hv", name="hv")
            nc.vector.tensor_tensor(hv[:], vv[:], hh[:], OP.add)
            # hyst = 1.25*hv > 1  <=>  hv > 0.8 (hv is a small integer count)
            hgt = tp.tile([PB, HB, W], BF16, tag="hgt", name="hgt")
            nc.vector.tensor_scalar(hgt[:], hv[:], 0.8, None, OP.is_gt)
            m5 = tp.tile([PB, HB, W], BF16, tag="m5", name="m5")
            nc.vector.tensor_scalar(m5[:], m[:], 0.5, None, OP.is_equal)
            w1 = tp.tile([PB, HB, W], BF16, tag="w1", name="w1")
            nc.vector.tensor_tensor(w1[:], hgt[:], m5[:], OP.mult)
            # w2 = w1*m, but w1 is only nonzero where m==0.5 exactly
            w2 = tp.tile([PB, HB, W], BF16, tag="w2", name="w2")
            nc.vector.tensor_scalar(w2[:], w1[:], 0.5, None, OP.mult)
            outb = tp.tile([PB, HB, W], BF16, tag="outb", name="outb")
            nc.vector.tensor_tensor(outb[:], w2[:], iSc[:], OP.add)
            outv = tp.tile([PB, HB, W], F32, tag="outv", name="outv")
            nc.scalar.activation(outv[:], outb[:], AF.Copy)
            nc.sync.dma_start(
                dram_ap(out_t, 0, [(W, PB), (PB * W, HB), (1, W)]), outv[:])

    nc.compile()
    return nc


_PROG_CACHE = {}


def _get_program(H, W, n_cores):
    key = (H, W, n_cores)
    if key not in _PROG_CACHE:
        _PROG_CACHE[key] = build_program(H, W, n_cores)
    return _PROG_CACHE[key]


def make_in_maps(x, w_expand, w_perm_gauss, w_perm_sx, w_perm_sy, w_merge,
                 low_t, high_t):
    B, Cin, H, W = x.shape
    wd = make_weights(np.asarray(w_expand, np.float64), np.asarray(w_perm_gauss, np.float64),
                      np.asarray(w_perm_sx, np.float64), np.asarray(w_perm_sy, np.float64),
                      np.asarray(w_merge, np.float64), np.asarray(low_t), np.asarray(high_t), H)
    xpad = np.pad(np.asarray(x, np.float32), ((0, 0), (0, 0), (0, 0), (2, 2)),
                  mode="reflect").astype(ml_dtypes.bfloat16)
    shared = {
        "wst": np.ascontiguousarray(wd["wst"]),
        "wredc": np.ascontiguousarray(wd["wredc"]),
        "wreds": np.ascontiguousarray(wd["wreds"]),
        "wshift": np.ascontiguousarray(wd["wshift"]),
        "l2v": wd["l2v"], "h2v": wd["h2v"],
        "zer": np.zeros((4, SCH * W), ml_dtypes.bfloat16),
    }
    return [dict(shared, xin=np.ascontiguousarray(xpad[b])) for b in range(B)]


def kernel(x, w_expand, w_perm_gauss, w_perm_sx, w_perm_sy, w_merge,
           low_t, high_t):
    x = np.asarray(x)
    B, Cin, H, W = x.shape
    assert (B, Cin) == (8, 3)
    nc = _get_program(H, W, 8)
    in_maps = make_in_maps(x, w_expand, w_perm_gauss, w_perm_sx, w_perm_sy,
                           w_merge, low_t, high_t)
    res = run_bass_kernel_spmd(nc, in_maps, core_ids=list(range(8)))
    out = np.stack([res.results[b]["out"] for b in range(8)])[:, None]
    return out.astype(np.float32)
